# revision 7
# baseline (speedup 1.0000x reference)
"""GCN encoder (2-layer GCNConv, PyG symmetric norm w/ self loops) on 8
Trainium2 NeuronCores.

  out = M @ relu(M @ x @ W1 + b1) @ W2 + b2,  M = D^-1/2 (A+I) D^-1/2

Design (v2):
- Node-sharded by destination (12.5k dests/core); every core gathers its
  edges' source rows with gpsimd dma_gather (4 SWDGE queues round-robin).
  Per-descriptor SDMA cost (~0.1us/desc/engine) dominates, so slot padding
  is minimized: slots are grouped per (super of 4 dest tiles, table block)
  with NO per-tile alignment; per-tile matmul block ranges are the UNION
  over cores (baked at compile time, shared SPMD program).
- The per-edge norm dis_i*dis_j is folded into the one-hot matrix:
  S[e, d] = norm_e * (dst_e == d), built by one DVE tensor_scalar
  (is_equal then mult; dst encoded 0..511 within the super, f32 iota).
- Layer 1 aggregates FEATURE-MAJOR: psT[f, d] += msgs[e, f-half].T @ S so
  the result feeds W1 (lhsT = W1 half) directly -- no transposes, no DRAM
  round trips. W1+relu+W2 run fused per dest tile right after aggregation;
  only the final h2 tile is PE-transposed back to node-major for the
  AllGather table.
- Layer 2 aggregates node-major from the AllGathered h2 table (plain
  values; norm again in S) and adds dis^2 * own h2 + b2.
"""

import math
import os

import numpy as np
import ml_dtypes

BF16 = ml_dtypes.bfloat16

# ---------------------------------------------------------------- problem cfg
N = 100000
E_EDGES = 3200000
IN_C = 256
HID = 256
OUT_C = 128
NCORES = 8
TSUP = 4  # dest tiles per super (dst encoded 0..TSUP*128-1)


def make_cfg(n_nodes, in_c, hid, out_c, msg_bufs=8):
    pn = n_nodes // NCORES
    tpc = (pn + 127) // 128
    ppad = tpc * 128
    cfg = dict(
        N=n_nodes,
        IN=in_c,
        HID=hid,
        OUT=out_c,
        PN=pn,
        TPC=tpc,
        PPAD=ppad,
        NB=4,
        BS1=n_nodes // 4,
        BS2=2 * ppad,
        MSG_BUFS=msg_bufs,
    )
    assert n_nodes % 8 == 0
    assert cfg["BS1"] < 32768 and cfg["BS2"] < 32768, "int16 gather index limit"
    return cfg


CFG = make_cfg(N, IN_C, HID, OUT_C)


# ---------------------------------------------------------------- host prep
def prep_inputs(cfg, x, edge_index, W1, b1, W2, b2):
    """Shard/encode on the host. Returns (in_maps, meta). The slot layout,
    segment sizes and per-tile block ranges are COMMON across cores (max /
    union over cores) so the compiled program is identical (SPMD)."""
    n, pn, tpc, ppad, nb = cfg["N"], cfg["PN"], cfg["TPC"], cfg["PPAD"], cfg["NB"]
    bs1 = cfg["BS1"]
    in_c, hid, out_c = cfg["IN"], cfg["HID"], cfg["OUT"]
    nsup = (tpc + TSUP - 1) // TSUP
    supers = [list(range(i, min(i + TSUP, tpc))) for i in range(0, tpc, TSUP)]

    x = np.asarray(x, np.float32)
    edge_index = np.asarray(edge_index)
    row = edge_index[0].astype(np.int64)
    col = edge_index[1].astype(np.int64)

    deg = np.bincount(col, minlength=n).astype(np.float32) + 1.0
    dis = 1.0 / np.sqrt(deg)  # [n]

    core = col // pn
    t_of = (col - core * pn) // 128
    si_of = t_of // TSUP
    tl_of = t_of % TSUP
    dls = (col - core * pn) % 128 + tl_of * 128  # dest encoded within super
    b_of = row // bs1
    l1i = (row - b_of * bs1).astype(np.int16)
    shard = row // pn
    l2i = ((shard - 2 * b_of) * ppad + (row - shard * pn)).astype(np.int16)
    norm = (dis[row] * dis[col]).astype(np.float32)

    # counts per (core, super, block, tile-in-super)
    key_t = (((core * nsup) + si_of) * nb + b_of) * TSUP + tl_of
    cnt = np.bincount(key_t, minlength=NCORES * nsup * nb * TSUP).reshape(
        NCORES, nsup, nb, TSUP
    )
    seg = cnt.sum(axis=3).max(axis=0)  # [nsup, nb] max over cores
    seg = ((seg + 127) // 128) * 128

    # global layout: (super, block) contiguous; gather call sizes 256-quantized
    co = np.zeros((nsup, nb), np.int64)
    pos = 0
    for si in range(nsup):
        for b in range(nb):
            co[si][b] = pos
            pos += int(seg[si][b])
    tot = pos
    nidx = seg  # [nsup, nb]
    nidx_call = ((nidx + 255) // 256) * 256
    tot_io = tot + 512  # zero tail so quantized idx reads stay in bounds

    # union per-tile block ranges [jlo, jhi) within each (si, b) segment
    lo = np.cumsum(cnt, axis=3) - cnt  # [c, si, b, tl] start within group
    hi = lo + cnt
    jlo = np.where(cnt > 0, lo // 128, 10 ** 9).min(axis=0)  # [nsup, nb, tl]
    jhi = np.where(cnt > 0, (hi + 127) // 128, -1).max(axis=0)
    have = (cnt.sum(axis=0) > 0)  # [nsup, nb, tl]

    # per-edge positions: sort by (core, si, b, tl) with source row as the
    # secondary key -- slots within a subgroup ascend by table row, so the
    # gather's DMA descriptors access HBM quasi-sequentially (row-buffer
    # locality) instead of randomly.
    order = np.lexsort((row, key_t))
    group_start = np.zeros(NCORES * nsup * nb * TSUP + 1, np.int64)
    np.cumsum(
        np.bincount(key_t, minlength=NCORES * nsup * nb * TSUP),
        out=group_start[1:],
    )
    ranks = np.empty(len(order), np.int64)
    ranks[order] = np.arange(len(order)) - group_start[key_t[order]]
    # start of (c, si, b, tl) slots within the common segment = lo[c,si,b,tl]
    pos_of_edge = (
        co[si_of, b_of]
        + lo[core, si_of, b_of, tl_of]
        + ranks
    )

    in_maps = []
    w1c = np.ascontiguousarray(W1.astype(BF16).reshape(2, 128, hid))
    w2c = np.ascontiguousarray(W2.astype(BF16).reshape(2, 128, out_c))
    b1c = np.ascontiguousarray(
        np.asarray(b1, np.float32).reshape(2, 128).T
    )  # [128, 2]
    b2r = np.ascontiguousarray(np.tile(np.asarray(b2, np.float32)[None, :], (128, 1)))
    iota512 = np.ascontiguousarray(
        np.tile(np.arange(TSUP * 128, dtype=np.float32)[None, :], (128, 1))
    )
    ident = np.eye(128, dtype=np.float32).astype(BF16)

    xt = np.ascontiguousarray(x.astype(BF16))  # plain gather table

    for c in range(NCORES):
        sel = core == c
        p = pos_of_edge[sel]
        idx1 = np.zeros(tot, np.int16)
        idx2 = np.zeros(tot, np.int16)
        dst = np.full(tot, -1.0, np.float32)
        nrm = np.zeros(tot, np.float32)
        idx1[p] = l1i[sel]
        idx2[p] = l2i[sel]
        dst[p] = dls[sel].astype(np.float32)
        nrm[p] = norm[sel]

        idx1 = np.concatenate([idx1, np.zeros(tot_io - tot, np.int16)])
        idx2 = np.concatenate([idx2, np.zeros(tot_io - tot, np.int16)])
        idx1_w = np.tile(np.ascontiguousarray(idx1.reshape(-1, 16).T), (8, 1))
        idx2_w = np.tile(np.ascontiguousarray(idx2.reshape(-1, 16).T), (8, 1))
        dst_w = np.ascontiguousarray(dst.reshape(-1, 128).T)
        nrm_w = np.ascontiguousarray(nrm.reshape(-1, 128).T)

        dis_own = np.ones(ppad, np.float32)
        dis_own[:pn] = dis[c * pn : (c + 1) * pn]
        diso_w = np.ascontiguousarray(dis_own.reshape(tpc, 128).T)

        xself = np.zeros((ppad, in_c), np.float32)
        xself[:pn] = (
            x[c * pn : (c + 1) * pn] * (dis[c * pn : (c + 1) * pn] ** 2)[:, None]
        )
        xself = np.ascontiguousarray(xself).astype(BF16)

        in_maps.append(
            dict(
                xt=xt,
                xself=xself,
                idx1=idx1_w,
                idx2=idx2_w,
                dst=dst_w,
                nrm=nrm_w,
                diso=diso_w,
                w1=w1c,
                w2=w2c,
                b1c=b1c,
                b2=b2r,
                iota=iota512,
                ident=ident,
            )
        )

    meta = dict(
        co=co, nidx=nidx, nidx_call=nidx_call, tot=tot, tot_io=tot_io,
        jlo=jlo, jhi=jhi, have=have, supers=supers, nsup=nsup,
    )
    return in_maps, meta


# ---------------------------------------------------------------- bass build
def build_program(cfg, meta):
    import concourse.mybir as mybir
    import concourse.tile as tile
    from contextlib import ExitStack

    f32 = mybir.dt.float32
    bf16 = mybir.dt.bfloat16
    i16 = mybir.dt.int16
    Alu = mybir.AluOpType
    Act = mybir.ActivationFunctionType

    n, pn, tpc, ppad, nb = cfg["N"], cfg["PN"], cfg["TPC"], cfg["PPAD"], cfg["NB"]
    bs1, bs2 = cfg["BS1"], cfg["BS2"]
    in_c, hid, out_c = cfg["IN"], cfg["HID"], cfg["OUT"]
    co, nidx, nidx_call = meta["co"], meta["nidx"], meta["nidx_call"]
    tot, tot_io = meta["tot"], meta["tot_io"]
    jlo, jhi, have = meta["jlo"], meta["jhi"], meta["have"]
    supers, nsup = meta["supers"], meta["nsup"]

    import concourse.bacc as bacc

    nqueues = int(os.environ.get("GCN_QUEUES", "4"))
    nc = bacc.Bacc(None, num_devices=NCORES, num_swdge_queues=nqueues)

    xt_d = nc.dram_tensor("xt", [n, in_c], bf16, kind="ExternalInput")
    xself_d = nc.dram_tensor("xself", [ppad, in_c], bf16, kind="ExternalInput")
    idx1_d = nc.dram_tensor("idx1", [128, tot_io // 16], i16, kind="ExternalInput")
    idx2_d = nc.dram_tensor("idx2", [128, tot_io // 16], i16, kind="ExternalInput")
    dst_d = nc.dram_tensor("dst", [128, tot // 128], f32, kind="ExternalInput")
    nrm_d = nc.dram_tensor("nrm", [128, tot // 128], f32, kind="ExternalInput")
    diso_d = nc.dram_tensor("diso", [128, tpc], f32, kind="ExternalInput")
    w1_d = nc.dram_tensor("w1", [2, 128, hid], bf16, kind="ExternalInput")
    w2_d = nc.dram_tensor("w2", [2, 128, out_c], bf16, kind="ExternalInput")
    b1c_d = nc.dram_tensor("b1c", [128, 2], f32, kind="ExternalInput")
    b2_d = nc.dram_tensor("b2", [128, out_c], f32, kind="ExternalInput")
    iota_d = nc.dram_tensor("iota", [128, TSUP * 128], f32, kind="ExternalInput")
    ident_d = nc.dram_tensor("ident", [128, 128], bf16, kind="ExternalInput")
    out_d = nc.dram_tensor("out", [ppad, out_c], f32, kind="ExternalOutput")

    tab_d = nc.dram_tensor("tab", [NCORES * ppad, out_c], bf16, addr_space="Shared")

    nreg_cache = {}
    stages = os.environ.get("GCN_STAGES", "all")
    reps = int(os.environ.get("GCN_REPS", "1"))
    maxcols = int(nidx_call.max()) // 128
    max_super_cols = int(nidx.sum(axis=1).max()) // 128
    max_idx_cols = int(
        max(
            (co[si][nb - 1] - co[si][0] + nidx_call[si][nb - 1])
            for si in range(nsup)
        )
    ) // 16

    with tile.TileContext(nc) as tc, ExitStack() as ctx:
        def nreg(v):
            if v not in nreg_cache:
                nreg_cache[v] = nc.gpsimd.to_reg(v)
            return nreg_cache[v]

        cpool = ctx.enter_context(tc.tile_pool(name="const", bufs=1))
        iota_t = cpool.tile([128, TSUP * 128], f32)
        nc.sync.dma_start(iota_t[:], iota_d[:])
        ident_t = cpool.tile([128, 128], bf16)
        nc.sync.dma_start(ident_t[:], ident_d[:])
        w1_t = cpool.tile([128, 2, hid], bf16)
        w2_t = cpool.tile([128, 2, out_c], bf16)
        for k in range(2):
            nc.sync.dma_start(w1_t[:, k, :], w1_d[k])
            nc.sync.dma_start(w2_t[:, k, :], w2_d[k])
        b1c_t = cpool.tile([128, 2], f32)
        nc.sync.dma_start(b1c_t[:], b1c_d[:])
        b2_t = cpool.tile([128, out_c], f32)
        nc.sync.dma_start(b2_t[:], b2_d[:])
        diso_t = cpool.tile([128, tpc], f32)
        nc.sync.dma_start(diso_t[:], diso_d[:])
        diso2_t = cpool.tile([128, tpc], f32)
        nc.vector.tensor_mul(diso2_t[:], diso_t[:], diso_t[:])

        gq = [0]

        def gather_super(si, idx_d, table_d, bs, feat, ipool, mpool):
            """Issue merged idx load + nb gathers for super si. Returns
            (msgs dict, gather insts)."""
            c0 = int(co[si][0])
            ncols = int(co[si][nb - 1] - c0 + nidx_call[si][nb - 1])
            it = ipool.tile([128, max_idx_cols], i16, tag="idx")
            nc.sync.dma_start(
                it[:, : ncols // 16], idx_d[:, c0 // 16 : (c0 + ncols) // 16]
            )
            msgs = {}
            gathers = []
            for b in range(nb):
                ni = int(nidx_call[si][b])
                if ni == 0:
                    continue
                o0 = int(co[si][b]) - c0
                mt = mpool.tile([128, maxcols, feat], bf16, tag="msgs")
                g = nc.gpsimd.dma_gather(
                    mt[:, : ni // 128, :],
                    table_d[b * bs : (b + 1) * bs, :],
                    it[:, o0 // 16 : (o0 + ni) // 16],
                    ni,
                    nreg(ni),
                    feat,
                    elem_step=feat,
                    single_packet=False,
                    queue_num=gq[0] % nqueues,
                )
                gq[0] += 1
                gathers.append(g)
                msgs[b] = mt
            return msgs, gathers

        def load_dstnrm(si, dpool):
            scol0 = int(co[si][0]) // 128
            scols = int(nidx[si].sum()) // 128
            dt_ = dpool.tile([128, max_super_cols], f32, tag="dst")
            nc.sync.dma_start(dt_[:, :scols], dst_d[:, scol0 : scol0 + scols])
            nm_ = dpool.tile([128, max_super_cols], f32, tag="nrm")
            nc.sync.dma_start(nm_[:, :scols], nrm_d[:, scol0 : scol0 + scols])
            return dt_, nm_

        def tile_blocks(si, tl):
            """(b, j, dcol) list for dest tile tl of super si (union range)."""
            out = []
            for b in range(nb):
                if not have[si][b][tl]:
                    continue
                base = (int(co[si][b]) - int(co[si][0])) // 128
                for j in range(int(jlo[si][b][tl]), int(jhi[si][b][tl])):
                    out.append((b, j, base + j))
            return out

        def build_S(spool, iota_ap, dt_, nm_, dcol):
            S = spool.tile([128, 128], bf16, tag="oneh")
            nc.vector.tensor_scalar(
                S[:],
                iota_ap,
                dt_[:, dcol : dcol + 1],
                nm_[:, dcol : dcol + 1],
                Alu.is_equal,
                Alu.mult,
            )
            return S

        def _one_pass():
            # ---------------- layer 1 + W1 + relu + W2, fused per super
            with (
                tc.tile_pool(name="idx1", bufs=3) as ipool,
                tc.tile_pool(name="msgs", bufs=cfg["MSG_BUFS"]) as mpool,
                tc.tile_pool(name="dstp", bufs=3) as dpool,
                tc.tile_pool(name="oneh", bufs=8) as spool,
                tc.tile_pool(name="xsl", bufs=3) as xsp,
                tc.tile_pool(name="mxT", bufs=6) as mxp,
                tc.tile_pool(name="o1T", bufs=8) as o1p,
                tc.tile_pool(name="h2s", bufs=8) as h2p,
                tc.tile_pool(name="psagg", bufs=3, space="PSUM") as pa,
                tc.tile_pool(name="psW", bufs=2, space="PSUM") as pw,
                tc.tile_pool(name="psW2", bufs=1, space="PSUM") as pw2,
                tc.tile_pool(name="psTr", bufs=2, space="PSUM") as ptr,
            ):
                def w_chain(pend):
                    """mx add + PE-transpose + W1 + relu + W2 + h2 transpose
                    for a finished tile (emitted after the NEXT tile's agg
                    matmuls so PE stays fed)."""
                    psf, xl, t = pend
                    mx = mxp.tile([128, in_c], bf16, tag="mxT")
                    nc.vector.tensor_add(mx[:], psf[:], xl)
                    mxT = []
                    for kk in range(2):
                        ptt = ptr.tile([128, 128], bf16, tag="psTr")
                        nc.tensor.transpose(
                            ptt[:], mx[:, kk * 128 : (kk + 1) * 128], ident_t[:]
                        )
                        mt = o1p.tile([128, 128], bf16, tag="mxTs")
                        nc.scalar.copy(mt[:], ptt[:])
                        mxT.append(mt)
                    ps1 = pw.tile([128, 256], f32, tag="psW")
                    o1T = []
                    for m in range(2):
                        for kk in range(2):
                            nc.tensor.matmul(
                                ps1[:, m * 128 : (m + 1) * 128],
                                w1_t[:, kk, m * 128 : (m + 1) * 128],
                                mxT[kk][:],
                                start=(kk == 0),
                                stop=(kk == 1),
                            )
                        o1 = o1p.tile([128, 128], bf16, tag="o1T")
                        nc.scalar.activation(
                            o1[:],
                            ps1[:, m * 128 : (m + 1) * 128],
                            Act.Relu,
                            bias=b1c_t[:, m : m + 1],
                        )
                        o1T.append(o1)
                    ps2 = pw2.tile([128, out_c], f32, tag="psW2")
                    for kk in range(2):
                        nc.tensor.matmul(
                            ps2[:],
                            w2_t[:, kk, :],
                            o1T[kk][:],
                            start=(kk == 0),
                            stop=(kk == 1),
                        )
                    h2T = h2p.tile([128, 128], bf16, tag="h2s")
                    nc.scalar.copy(h2T[:], ps2[:])
                    pst = ptr.tile([128, 128], bf16, tag="psTr")
                    nc.tensor.transpose(pst[:], h2T[:], ident_t[:])
                    h2 = h2p.tile([128, 128], bf16, tag="h2s")
                    nc.vector.tensor_copy(h2[:], pst[:])
                    nc.sync.dma_start(agin_dm[t * 128 : (t + 1) * 128, :], h2[:])

                pend = None
                for si, s in enumerate(supers):
                    msgs, _ = gather_super(si, idx1_d, xt_d, bs1, in_c, ipool, mpool)
                    dt_, nm_ = load_dstnrm(si, dpool)
                    xls = {}
                    for tl, t in enumerate(s):
                        xl = xsp.tile([128, in_c], bf16, tag="xsl")
                        nc.sync.dma_start(
                            xl[:], xself_d[t * 128 : (t + 1) * 128, :]
                        )
                        xls[tl] = xl
                    for tl, t in enumerate(s):
                        blks = tile_blocks(si, tl)
                        assert blks, f"empty tile {si}/{tl}"
                        iota_ap = iota_t[:, tl * 128 : (tl + 1) * 128]
                        if os.environ.get("GCN_SKIP_MM"):
                            psf = None
                        else:
                            psf = pa.tile([128, in_c], f32, tag="psagg")
                        nblk = len(blks)
                        for k, (b, j, dcol) in enumerate(blks):
                            S = build_S(spool, iota_ap, dt_, nm_, dcol)
                            if os.environ.get("GCN_SKIP_MM"):
                                continue
                            nc.tensor.matmul(
                                psf[:],
                                S[:],
                                msgs[b][:, j, :],
                                start=(k == 0),
                                stop=(k == nblk - 1),
                            )
                        if os.environ.get("GCN_SKIP_MM") or os.environ.get("GCN_SKIP_W"):
                            continue
                        if pend is not None:
                            w_chain(pend)
                        pend = (psf, xls[tl][:], t)
                if pend is not None:
                    w_chain(pend)

            # ---------------- AllGather h2 table
            if stages in ("ag", "all"):
                cc = nc.gpsimd.collective_compute(
                    "AllGather",
                    mybir.AluOpType.bypass,
                    replica_groups=[list(range(NCORES))],
                    ins=[agin_dm[:, :].opt()],
                    outs=[tab_d.ap().opt()],
                )

            # ---------------- layer-2 aggregation + epilogue
            import bass_rust as _br

            if stages == "all":
                with (
                    tc.tile_pool(name="idx2", bufs=3) as ipool2,
                    tc.tile_pool(name="msgs2", bufs=cfg["MSG_BUFS"]) as mpool2,
                    tc.tile_pool(name="dstp2", bufs=3) as dpool2,
                    tc.tile_pool(name="oneh2", bufs=8) as spool2,
                    tc.tile_pool(name="h2own", bufs=10) as hop,
                    tc.tile_pool(name="wtmp", bufs=6) as wtp,
                    tc.tile_pool(name="o2", bufs=6) as o2p,
                    tc.tile_pool(name="psag2", bufs=6, space="PSUM") as pa2,
                ):
                    for si, s in enumerate(supers):
                        msgs, gs = gather_super(
                            si, idx2_d, tab_d, bs2, out_c, ipool2, mpool2
                        )
                        for g in gs:
                            _br.add_dep_helper(
                                g.ins, cc.ins, sync=True, reason="tab after AG"
                            )
                        dt_, nm_ = load_dstnrm(si, dpool2)
                        h2os = {}
                        for tl, t in enumerate(s):
                            h2o = hop.tile([128, out_c], bf16, tag="h2own")
                            nc.sync.dma_start(
                                h2o[:], agin_dm[t * 128 : (t + 1) * 128, :]
                            )
                            h2os[tl] = h2o[:]
                        for tl, t in enumerate(s):
                            blks = tile_blocks(si, tl)
                            iota_ap = iota_t[:, tl * 128 : (tl + 1) * 128]
                            ps = pa2.tile([128, out_c], f32, tag="psag2")
                            nblk = len(blks)
                            for k, (b, j, dcol) in enumerate(blks):
                                S = build_S(spool2, iota_ap, dt_, nm_, dcol)
                                nc.tensor.matmul(
                                    ps[:],
                                    S[:],
                                    msgs[b][:, j, :],
                                    start=(k == 0),
                                    stop=(k == nblk - 1),
                                )
                            wt = wtp.tile([128, out_c], f32, tag="wtmp")
                            nc.vector.scalar_tensor_tensor(
                                wt[:],
                                h2os[tl],
                                diso2_t[:, t : t + 1],
                                b2_t[:],
                                Alu.mult,
                                Alu.add,
                            )
                            o2 = o2p.tile([128, out_c], f32, tag="o2")
                            nc.vector.tensor_add(o2[:], ps[:], wt[:])
                            nc.sync.dma_start(
                                out_d[t * 128 : (t + 1) * 128, :], o2[:]
                            )

        dram = ctx.enter_context(tc.tile_pool(name="dram", bufs=1, space="DRAM"))
        agin_dm = dram.tile([ppad, out_c], bf16)

        for _ in range(reps):
            _one_pass()

        if stages == "dbg":
            nc.gpsimd.dma_start(out_d[:, :], agin_dm[:, :])
        elif stages != "all":
            nc.sync.dma_start(out_d[0:128, :], b2_t[:])

    nc.compile()
    return nc


# ---------------------------------------------------------------- entry point
def kernel(x, edge_index, W1, b1, W2, b2):
    from concourse.bass_utils import run_bass_kernel_spmd

    cfg = CFG
    in_maps, meta = prep_inputs(cfg, x, edge_index, W1, b1, W2, b2)
    nc = build_program(cfg, meta)
    res = run_bass_kernel_spmd(nc, in_maps, core_ids=list(range(NCORES)))
    outs = [r["out"][: cfg["PN"]] for r in res.results]
    return np.concatenate(outs, axis=0).astype(np.float32)



# revision 9
# speedup vs baseline: 1.0016x; 1.0016x over previous
"""GCN encoder (2-layer GCNConv, PyG symmetric norm w/ self loops) on 8
Trainium2 NeuronCores.

  out = M @ relu(M @ x @ W1 + b1) @ W2 + b2,  M = D^-1/2 (A+I) D^-1/2

Design (v2):
- Node-sharded by destination (12.5k dests/core); every core gathers its
  edges' source rows with gpsimd dma_gather (4 SWDGE queues round-robin).
  Per-descriptor SDMA cost (~0.1us/desc/engine) dominates, so slot padding
  is minimized: slots are grouped per (super of 4 dest tiles, table block)
  with NO per-tile alignment; per-tile matmul block ranges are the UNION
  over cores (baked at compile time, shared SPMD program).
- The per-edge norm dis_i*dis_j is folded into the one-hot matrix:
  S[e, d] = norm_e * (dst_e == d), built by one DVE tensor_scalar
  (is_equal then mult; dst encoded 0..511 within the super, f32 iota).
- Layer 1 aggregates FEATURE-MAJOR: psT[f, d] += msgs[e, f-half].T @ S so
  the result feeds W1 (lhsT = W1 half) directly -- no transposes, no DRAM
  round trips. W1+relu+W2 run fused per dest tile right after aggregation;
  only the final h2 tile is PE-transposed back to node-major for the
  AllGather table.
- Layer 2 aggregates node-major from the AllGathered h2 table (plain
  values; norm again in S) and adds dis^2 * own h2 + b2.
"""

import math
import os

import numpy as np
import ml_dtypes

BF16 = ml_dtypes.bfloat16

# ---------------------------------------------------------------- problem cfg
N = 100000
E_EDGES = 3200000
IN_C = 256
HID = 256
OUT_C = 128
NCORES = 8
TSUP = 4  # dest tiles per super (dst encoded 0..TSUP*128-1)


def make_cfg(n_nodes, in_c, hid, out_c, msg_bufs=8):
    pn = n_nodes // NCORES
    tpc = (pn + 127) // 128
    ppad = tpc * 128
    cfg = dict(
        N=n_nodes,
        IN=in_c,
        HID=hid,
        OUT=out_c,
        PN=pn,
        TPC=tpc,
        PPAD=ppad,
        NB=4,
        BS1=n_nodes // 4,
        BS2=2 * ppad,
        MSG_BUFS=msg_bufs,
    )
    assert n_nodes % 8 == 0
    assert cfg["BS1"] < 32768 and cfg["BS2"] < 32768, "int16 gather index limit"
    return cfg


CFG = make_cfg(N, IN_C, HID, OUT_C)


# ---------------------------------------------------------------- host prep
def prep_inputs(cfg, x, edge_index, W1, b1, W2, b2):
    """Shard/encode on the host. Returns (in_maps, meta). The slot layout,
    segment sizes and per-tile block ranges are COMMON across cores (max /
    union over cores) so the compiled program is identical (SPMD)."""
    n, pn, tpc, ppad, nb = cfg["N"], cfg["PN"], cfg["TPC"], cfg["PPAD"], cfg["NB"]
    bs1 = cfg["BS1"]
    in_c, hid, out_c = cfg["IN"], cfg["HID"], cfg["OUT"]
    nsup = (tpc + TSUP - 1) // TSUP
    supers = [list(range(i, min(i + TSUP, tpc))) for i in range(0, tpc, TSUP)]

    x = np.asarray(x, np.float32)
    edge_index = np.asarray(edge_index)
    row = edge_index[0].astype(np.int64)
    col = edge_index[1].astype(np.int64)

    deg = np.bincount(col, minlength=n).astype(np.float32) + 1.0
    dis = 1.0 / np.sqrt(deg)  # [n]

    core = col // pn
    t_of = (col - core * pn) // 128
    si_of = t_of // TSUP
    tl_of = t_of % TSUP
    dls = (col - core * pn) % 128 + tl_of * 128  # dest encoded within super
    b_of = row // bs1
    l1i = (row - b_of * bs1).astype(np.int16)
    shard = row // pn
    l2i = ((shard - 2 * b_of) * ppad + (row - shard * pn)).astype(np.int16)
    norm = (dis[row] * dis[col]).astype(np.float32)

    # counts per (core, super, block, tile-in-super)
    key_t = (((core * nsup) + si_of) * nb + b_of) * TSUP + tl_of
    cnt = np.bincount(key_t, minlength=NCORES * nsup * nb * TSUP).reshape(
        NCORES, nsup, nb, TSUP
    )
    seg = cnt.sum(axis=3).max(axis=0)  # [nsup, nb] max over cores
    seg = ((seg + 127) // 128) * 128

    # global layout: (super, block) contiguous; gather call sizes 256-quantized
    co = np.zeros((nsup, nb), np.int64)
    pos = 0
    for si in range(nsup):
        for b in range(nb):
            co[si][b] = pos
            pos += int(seg[si][b])
    tot = pos
    nidx = seg  # [nsup, nb]
    nidx_call = ((nidx + 255) // 256) * 256
    tot_io = tot + 512  # zero tail so quantized idx reads stay in bounds

    # union per-tile block ranges [jlo, jhi) within each (si, b) segment
    lo = np.cumsum(cnt, axis=3) - cnt  # [c, si, b, tl] start within group
    hi = lo + cnt
    jlo = np.where(cnt > 0, lo // 128, 10 ** 9).min(axis=0)  # [nsup, nb, tl]
    jhi = np.where(cnt > 0, (hi + 127) // 128, -1).max(axis=0)
    have = (cnt.sum(axis=0) > 0)  # [nsup, nb, tl]

    # per-edge positions: sort by (core, si, b, tl) with source row as the
    # secondary key -- slots within a subgroup ascend by table row, so the
    # gather's DMA descriptors access HBM quasi-sequentially (row-buffer
    # locality) instead of randomly.
    order = np.lexsort((row, key_t))
    group_start = np.zeros(NCORES * nsup * nb * TSUP + 1, np.int64)
    np.cumsum(
        np.bincount(key_t, minlength=NCORES * nsup * nb * TSUP),
        out=group_start[1:],
    )
    ranks = np.empty(len(order), np.int64)
    ranks[order] = np.arange(len(order)) - group_start[key_t[order]]
    # start of (c, si, b, tl) slots within the common segment = lo[c,si,b,tl]
    pos_of_edge = (
        co[si_of, b_of]
        + lo[core, si_of, b_of, tl_of]
        + ranks
    )

    in_maps = []
    w1c = np.ascontiguousarray(W1.astype(BF16).reshape(2, 128, hid))
    w2c = np.ascontiguousarray(W2.astype(BF16).reshape(2, 128, out_c))
    b1c = np.ascontiguousarray(
        np.asarray(b1, np.float32).reshape(2, 128).T
    )  # [128, 2]
    b2r = np.ascontiguousarray(np.tile(np.asarray(b2, np.float32)[None, :], (128, 1)))
    iota512 = np.ascontiguousarray(
        np.tile(np.arange(TSUP * 128, dtype=np.float32)[None, :], (128, 1))
    )
    ident = np.eye(128, dtype=np.float32).astype(BF16)

    xt = np.ascontiguousarray(x.astype(BF16))  # plain gather table

    for c in range(NCORES):
        sel = core == c
        p = pos_of_edge[sel]
        idx1 = np.zeros(tot, np.int16)
        idx2 = np.zeros(tot, np.int16)
        dst = np.full(tot, -1.0, np.float32)
        nrm = np.zeros(tot, np.float32)
        idx1[p] = l1i[sel]
        idx2[p] = l2i[sel]
        dst[p] = dls[sel].astype(np.float32)
        nrm[p] = norm[sel]

        idx1 = np.concatenate([idx1, np.zeros(tot_io - tot, np.int16)])
        idx2 = np.concatenate([idx2, np.zeros(tot_io - tot, np.int16)])
        if os.environ.get("GCN_CONST_IDX"):
            idx1[:] = 0
            idx2[:] = 0
        idx1_w = np.tile(np.ascontiguousarray(idx1.reshape(-1, 16).T), (8, 1))
        idx2_w = np.tile(np.ascontiguousarray(idx2.reshape(-1, 16).T), (8, 1))
        dst_w = np.ascontiguousarray(dst.reshape(-1, 128).T)
        nrm_w = np.ascontiguousarray(nrm.reshape(-1, 128).T)

        dis_own = np.ones(ppad, np.float32)
        dis_own[:pn] = dis[c * pn : (c + 1) * pn]
        diso_w = np.ascontiguousarray(dis_own.reshape(tpc, 128).T)

        xself = np.zeros((ppad, in_c), np.float32)
        xself[:pn] = (
            x[c * pn : (c + 1) * pn] * (dis[c * pn : (c + 1) * pn] ** 2)[:, None]
        )
        xself = np.ascontiguousarray(xself).astype(BF16)

        in_maps.append(
            dict(
                xt=xt,
                xself=xself,
                idx1=idx1_w,
                idx2=idx2_w,
                dst=dst_w,
                nrm=nrm_w,
                diso=diso_w,
                w1=w1c,
                w2=w2c,
                b1c=b1c,
                b2=b2r,
                iota=iota512,
                ident=ident,
            )
        )

    meta = dict(
        co=co, nidx=nidx, nidx_call=nidx_call, tot=tot, tot_io=tot_io,
        jlo=jlo, jhi=jhi, have=have, supers=supers, nsup=nsup,
    )
    return in_maps, meta


# ---------------------------------------------------------------- bass build
def build_program(cfg, meta):
    import concourse.mybir as mybir
    import concourse.tile as tile
    from contextlib import ExitStack

    f32 = mybir.dt.float32
    bf16 = mybir.dt.bfloat16
    i16 = mybir.dt.int16
    Alu = mybir.AluOpType
    Act = mybir.ActivationFunctionType

    n, pn, tpc, ppad, nb = cfg["N"], cfg["PN"], cfg["TPC"], cfg["PPAD"], cfg["NB"]
    bs1, bs2 = cfg["BS1"], cfg["BS2"]
    in_c, hid, out_c = cfg["IN"], cfg["HID"], cfg["OUT"]
    co, nidx, nidx_call = meta["co"], meta["nidx"], meta["nidx_call"]
    tot, tot_io = meta["tot"], meta["tot_io"]
    jlo, jhi, have = meta["jlo"], meta["jhi"], meta["have"]
    supers, nsup = meta["supers"], meta["nsup"]

    import concourse.bacc as bacc

    nqueues = int(os.environ.get("GCN_QUEUES", "4"))
    nc = bacc.Bacc(None, num_devices=NCORES, num_swdge_queues=nqueues)

    xt_d = nc.dram_tensor("xt", [n, in_c], bf16, kind="ExternalInput")
    xself_d = nc.dram_tensor("xself", [ppad, in_c], bf16, kind="ExternalInput")
    idx1_d = nc.dram_tensor("idx1", [128, tot_io // 16], i16, kind="ExternalInput")
    idx2_d = nc.dram_tensor("idx2", [128, tot_io // 16], i16, kind="ExternalInput")
    dst_d = nc.dram_tensor("dst", [128, tot // 128], f32, kind="ExternalInput")
    nrm_d = nc.dram_tensor("nrm", [128, tot // 128], f32, kind="ExternalInput")
    diso_d = nc.dram_tensor("diso", [128, tpc], f32, kind="ExternalInput")
    w1_d = nc.dram_tensor("w1", [2, 128, hid], bf16, kind="ExternalInput")
    w2_d = nc.dram_tensor("w2", [2, 128, out_c], bf16, kind="ExternalInput")
    b1c_d = nc.dram_tensor("b1c", [128, 2], f32, kind="ExternalInput")
    b2_d = nc.dram_tensor("b2", [128, out_c], f32, kind="ExternalInput")
    iota_d = nc.dram_tensor("iota", [128, TSUP * 128], f32, kind="ExternalInput")
    ident_d = nc.dram_tensor("ident", [128, 128], bf16, kind="ExternalInput")
    out_d = nc.dram_tensor("out", [ppad, out_c], f32, kind="ExternalOutput")

    tab_d = nc.dram_tensor("tab", [NCORES * ppad, out_c], bf16, addr_space="Shared")

    nreg_cache = {}
    stages = os.environ.get("GCN_STAGES", "all")
    reps = int(os.environ.get("GCN_REPS", "1"))
    maxcols = int(nidx_call.max()) // 128
    max_super_cols = int(nidx.sum(axis=1).max()) // 128
    max_idx_cols = int(
        max(
            (co[si][nb - 1] - co[si][0] + nidx_call[si][nb - 1])
            for si in range(nsup)
        )
    ) // 16

    with tile.TileContext(nc) as tc, ExitStack() as ctx:
        def nreg(v):
            if v not in nreg_cache:
                nreg_cache[v] = nc.gpsimd.to_reg(v)
            return nreg_cache[v]

        cpool = ctx.enter_context(tc.tile_pool(name="const", bufs=1))
        iota_t = cpool.tile([128, TSUP * 128], f32)
        nc.sync.dma_start(iota_t[:], iota_d[:])
        ident_t = cpool.tile([128, 128], bf16)
        nc.sync.dma_start(ident_t[:], ident_d[:])
        w1_t = cpool.tile([128, 2, hid], bf16)
        w2_t = cpool.tile([128, 2, out_c], bf16)
        for k in range(2):
            nc.sync.dma_start(w1_t[:, k, :], w1_d[k])
            nc.sync.dma_start(w2_t[:, k, :], w2_d[k])
        b1c_t = cpool.tile([128, 2], f32)
        nc.sync.dma_start(b1c_t[:], b1c_d[:])
        b2_t = cpool.tile([128, out_c], f32)
        nc.sync.dma_start(b2_t[:], b2_d[:])
        diso_t = cpool.tile([128, tpc], f32)
        nc.sync.dma_start(diso_t[:], diso_d[:])
        diso2_t = cpool.tile([128, tpc], f32)
        nc.vector.tensor_mul(diso2_t[:], diso_t[:], diso_t[:])

        gq = [0]

        def gather_super(si, idx_d, table_d, bs, feat, ipool, mpool):
            """Issue merged idx load + nb gathers for super si. Returns
            (msgs dict, gather insts)."""
            c0 = int(co[si][0])
            ncols = int(co[si][nb - 1] - c0 + nidx_call[si][nb - 1])
            it = ipool.tile([128, max_idx_cols], i16, tag="idx")
            nc.sync.dma_start(
                it[:, : ncols // 16], idx_d[:, c0 // 16 : (c0 + ncols) // 16]
            )
            msgs = {}
            gathers = []
            for b in range(nb):
                ni = int(nidx_call[si][b])
                if ni == 0:
                    continue
                o0 = int(co[si][b]) - c0
                mt = mpool.tile([128, maxcols, feat], bf16, tag="msgs")
                if os.environ.get("GCN_SKIP_GATHER"):
                    msgs[b] = mt
                    continue
                g = nc.gpsimd.dma_gather(
                    mt[:, : ni // 128, :],
                    table_d[b * bs : (b + 1) * bs, :],
                    it[:, o0 // 16 : (o0 + ni) // 16],
                    ni,
                    nreg(ni),
                    feat,
                    elem_step=feat,
                    single_packet=False,
                    queue_num=gq[0] % nqueues,
                )
                gq[0] += 1
                gathers.append(g)
                msgs[b] = mt
            return msgs, gathers

        def load_dstnrm(si, dpool):
            scol0 = int(co[si][0]) // 128
            scols = int(nidx[si].sum()) // 128
            dt_ = dpool.tile([128, max_super_cols], f32, tag="dst")
            nc.sync.dma_start(dt_[:, :scols], dst_d[:, scol0 : scol0 + scols])
            nm_ = dpool.tile([128, max_super_cols], f32, tag="nrm")
            nc.sync.dma_start(nm_[:, :scols], nrm_d[:, scol0 : scol0 + scols])
            return dt_, nm_

        def tile_blocks(si, tl):
            """(b, j, dcol) list for dest tile tl of super si (union range)."""
            out = []
            for b in range(nb):
                if not have[si][b][tl]:
                    continue
                base = (int(co[si][b]) - int(co[si][0])) // 128
                for j in range(int(jlo[si][b][tl]), int(jhi[si][b][tl])):
                    out.append((b, j, base + j))
            return out

        def build_S(spool, iota_ap, dt_, nm_, dcol):
            S = spool.tile([128, 128], bf16, tag="oneh")
            nc.vector.tensor_scalar(
                S[:],
                iota_ap,
                dt_[:, dcol : dcol + 1],
                nm_[:, dcol : dcol + 1],
                Alu.is_equal,
                Alu.mult,
            )
            return S

        def _one_pass():
            # ---------------- layer 1 + W1 + relu + W2, fused per super
            with (
                tc.tile_pool(name="idx1", bufs=3) as ipool,
                tc.tile_pool(name="msgs", bufs=cfg["MSG_BUFS"]) as mpool,
                tc.tile_pool(name="dstp", bufs=3) as dpool,
                tc.tile_pool(name="oneh", bufs=8) as spool,
                tc.tile_pool(name="xsl", bufs=3) as xsp,
                tc.tile_pool(name="mxT", bufs=6) as mxp,
                tc.tile_pool(name="o1T", bufs=8) as o1p,
                tc.tile_pool(name="h2s", bufs=8) as h2p,
                tc.tile_pool(name="psagg", bufs=3, space="PSUM") as pa,
                tc.tile_pool(name="psW", bufs=2, space="PSUM") as pw,
                tc.tile_pool(name="psW2", bufs=1, space="PSUM") as pw2,
                tc.tile_pool(name="psTr", bufs=2, space="PSUM") as ptr,
            ):
                def w_chain(pend):
                    """mx add + PE-transpose + W1 + relu + W2 + h2 transpose
                    for a finished tile (emitted after the NEXT tile's agg
                    matmuls so PE stays fed)."""
                    psf, xl, t = pend
                    mx = mxp.tile([128, in_c], bf16, tag="mxT")
                    nc.vector.tensor_add(mx[:], psf[:], xl)
                    mxT = []
                    for kk in range(2):
                        ptt = ptr.tile([128, 128], bf16, tag="psTr")
                        nc.tensor.transpose(
                            ptt[:], mx[:, kk * 128 : (kk + 1) * 128], ident_t[:]
                        )
                        mt = o1p.tile([128, 128], bf16, tag="mxTs")
                        nc.scalar.copy(mt[:], ptt[:])
                        mxT.append(mt)
                    ps1 = pw.tile([128, 256], f32, tag="psW")
                    o1T = []
                    for m in range(2):
                        for kk in range(2):
                            nc.tensor.matmul(
                                ps1[:, m * 128 : (m + 1) * 128],
                                w1_t[:, kk, m * 128 : (m + 1) * 128],
                                mxT[kk][:],
                                start=(kk == 0),
                                stop=(kk == 1),
                            )
                        o1 = o1p.tile([128, 128], bf16, tag="o1T")
                        nc.scalar.activation(
                            o1[:],
                            ps1[:, m * 128 : (m + 1) * 128],
                            Act.Relu,
                            bias=b1c_t[:, m : m + 1],
                        )
                        o1T.append(o1)
                    ps2 = pw2.tile([128, out_c], f32, tag="psW2")
                    for kk in range(2):
                        nc.tensor.matmul(
                            ps2[:],
                            w2_t[:, kk, :],
                            o1T[kk][:],
                            start=(kk == 0),
                            stop=(kk == 1),
                        )
                    h2T = h2p.tile([128, 128], bf16, tag="h2s")
                    nc.scalar.copy(h2T[:], ps2[:])
                    pst = ptr.tile([128, 128], bf16, tag="psTr")
                    nc.tensor.transpose(pst[:], h2T[:], ident_t[:])
                    h2 = h2p.tile([128, 128], bf16, tag="h2s")
                    nc.vector.tensor_copy(h2[:], pst[:])
                    nc.sync.dma_start(agin_dm[t * 128 : (t + 1) * 128, :], h2[:])

                pend = None
                for si, s in enumerate(supers):
                    msgs, _ = gather_super(si, idx1_d, xt_d, bs1, in_c, ipool, mpool)
                    dt_, nm_ = load_dstnrm(si, dpool)
                    xls = {}
                    for tl, t in enumerate(s):
                        xl = xsp.tile([128, in_c], bf16, tag="xsl")
                        nc.sync.dma_start(
                            xl[:], xself_d[t * 128 : (t + 1) * 128, :]
                        )
                        xls[tl] = xl
                    for tl, t in enumerate(s):
                        blks = tile_blocks(si, tl)
                        assert blks, f"empty tile {si}/{tl}"
                        iota_ap = iota_t[:, tl * 128 : (tl + 1) * 128]
                        if os.environ.get("GCN_SKIP_MM"):
                            psf = None
                        else:
                            psf = pa.tile([128, in_c], f32, tag="psagg")
                        nblk = len(blks)
                        for k, (b, j, dcol) in enumerate(blks):
                            S = build_S(spool, iota_ap, dt_, nm_, dcol)
                            if os.environ.get("GCN_SKIP_MM"):
                                continue
                            nc.tensor.matmul(
                                psf[:],
                                S[:],
                                msgs[b][:, j, :],
                                start=(k == 0),
                                stop=(k == nblk - 1),
                            )
                        if os.environ.get("GCN_SKIP_MM") or os.environ.get("GCN_SKIP_W"):
                            continue
                        if pend is not None:
                            w_chain(pend)
                        pend = (psf, xls[tl][:], t)
                if pend is not None:
                    w_chain(pend)

            # ---------------- AllGather h2 table
            if stages in ("ag", "all"):
                cc = nc.gpsimd.collective_compute(
                    "AllGather",
                    mybir.AluOpType.bypass,
                    replica_groups=[list(range(NCORES))],
                    ins=[agin_dm[:, :].opt()],
                    outs=[tab_d.ap().opt()],
                )

            # ---------------- layer-2 aggregation + epilogue
            import bass_rust as _br

            if stages == "all":
                with (
                    tc.tile_pool(name="idx2", bufs=3) as ipool2,
                    tc.tile_pool(name="msgs2", bufs=cfg["MSG_BUFS"]) as mpool2,
                    tc.tile_pool(name="dstp2", bufs=3) as dpool2,
                    tc.tile_pool(name="oneh2", bufs=8) as spool2,
                    tc.tile_pool(name="h2own", bufs=10) as hop,
                    tc.tile_pool(name="wtmp", bufs=6) as wtp,
                    tc.tile_pool(name="o2", bufs=6) as o2p,
                    tc.tile_pool(name="psag2", bufs=6, space="PSUM") as pa2,
                ):
                    for si, s in enumerate(supers):
                        msgs, gs = gather_super(
                            si, idx2_d, tab_d, bs2, out_c, ipool2, mpool2
                        )
                        for g in gs:
                            _br.add_dep_helper(
                                g.ins, cc.ins, sync=True, reason="tab after AG"
                            )
                        dt_, nm_ = load_dstnrm(si, dpool2)
                        h2os = {}
                        for tl, t in enumerate(s):
                            h2o = hop.tile([128, out_c], bf16, tag="h2own")
                            nc.sync.dma_start(
                                h2o[:], agin_dm[t * 128 : (t + 1) * 128, :]
                            )
                            h2os[tl] = h2o[:]
                        for tl, t in enumerate(s):
                            blks = tile_blocks(si, tl)
                            iota_ap = iota_t[:, tl * 128 : (tl + 1) * 128]
                            ps = pa2.tile([128, out_c], f32, tag="psag2")
                            nblk = len(blks)
                            for k, (b, j, dcol) in enumerate(blks):
                                S = build_S(spool2, iota_ap, dt_, nm_, dcol)
                                nc.tensor.matmul(
                                    ps[:],
                                    S[:],
                                    msgs[b][:, j, :],
                                    start=(k == 0),
                                    stop=(k == nblk - 1),
                                )
                            wt = wtp.tile([128, out_c], f32, tag="wtmp")
                            nc.vector.scalar_tensor_tensor(
                                wt[:],
                                h2os[tl],
                                diso2_t[:, t : t + 1],
                                b2_t[:],
                                Alu.mult,
                                Alu.add,
                            )
                            o2 = o2p.tile([128, out_c], f32, tag="o2")
                            nc.vector.tensor_add(o2[:], ps[:], wt[:])
                            nc.sync.dma_start(
                                out_d[t * 128 : (t + 1) * 128, :], o2[:]
                            )

        dram = ctx.enter_context(tc.tile_pool(name="dram", bufs=1, space="DRAM"))
        agin_dm = dram.tile([ppad, out_c], bf16)

        for _ in range(reps):
            _one_pass()

        if stages == "dbg":
            nc.gpsimd.dma_start(out_d[:, :], agin_dm[:, :])
        elif stages != "all":
            nc.sync.dma_start(out_d[0:128, :], b2_t[:])

    nc.compile()
    return nc


# ---------------------------------------------------------------- entry point
def kernel(x, edge_index, W1, b1, W2, b2):
    from concourse.bass_utils import run_bass_kernel_spmd

    cfg = CFG
    in_maps, meta = prep_inputs(cfg, x, edge_index, W1, b1, W2, b2)
    nc = build_program(cfg, meta)
    res = run_bass_kernel_spmd(nc, in_maps, core_ids=list(range(NCORES)))
    outs = [r["out"][: cfg["PN"]] for r in res.results]
    return np.concatenate(outs, axis=0).astype(np.float32)



# revision 10
# speedup vs baseline: 1.0696x; 1.0679x over previous
"""GCN encoder (2-layer GCNConv, PyG symmetric norm w/ self loops) on 8
Trainium2 NeuronCores.

  out = M @ relu(M @ x @ W1 + b1) @ W2 + b2,  M = D^-1/2 (A+I) D^-1/2

Design (v2):
- Node-sharded by destination (12.5k dests/core); every core gathers its
  edges' source rows with gpsimd dma_gather (4 SWDGE queues round-robin).
  Per-descriptor SDMA cost (~0.1us/desc/engine) dominates, so slot padding
  is minimized: slots are grouped per (super of 4 dest tiles, table block)
  with NO per-tile alignment; per-tile matmul block ranges are the UNION
  over cores (baked at compile time, shared SPMD program).
- The per-edge norm dis_i*dis_j is folded into the one-hot matrix:
  S[e, d] = norm_e * (dst_e == d), built by one DVE tensor_scalar
  (is_equal then mult; dst encoded 0..511 within the super, f32 iota).
- Layer 1 aggregates FEATURE-MAJOR: psT[f, d] += msgs[e, f-half].T @ S so
  the result feeds W1 (lhsT = W1 half) directly -- no transposes, no DRAM
  round trips. W1+relu+W2 run fused per dest tile right after aggregation;
  only the final h2 tile is PE-transposed back to node-major for the
  AllGather table.
- Layer 2 aggregates node-major from the AllGathered h2 table (plain
  values; norm again in S) and adds dis^2 * own h2 + b2.
"""

import math
import os

import numpy as np
import ml_dtypes

BF16 = ml_dtypes.bfloat16

# ---------------------------------------------------------------- problem cfg
N = 100000
E_EDGES = 3200000
IN_C = 256
HID = 256
OUT_C = 128
NCORES = 8
TSUP = 4  # dest tiles per super (dst encoded 0..TSUP*128-1)


def make_cfg(n_nodes, in_c, hid, out_c, msg_bufs=8):
    pn = n_nodes // NCORES
    tpc = (pn + 127) // 128
    ppad = tpc * 128
    cfg = dict(
        N=n_nodes,
        IN=in_c,
        HID=hid,
        OUT=out_c,
        PN=pn,
        TPC=tpc,
        PPAD=ppad,
        NB=4,
        BS1=n_nodes // 4,
        BS2=2 * ppad,
        MSG_BUFS=msg_bufs,
    )
    assert n_nodes % 8 == 0
    assert cfg["BS1"] < 32768 and cfg["BS2"] < 32768, "int16 gather index limit"
    return cfg


CFG = make_cfg(N, IN_C, HID, OUT_C)


# ---------------------------------------------------------------- host prep
def prep_inputs(cfg, x, edge_index, W1, b1, W2, b2):
    """Shard/encode on the host. Returns (in_maps, meta). The slot layout,
    segment sizes and per-tile block ranges are COMMON across cores (max /
    union over cores) so the compiled program is identical (SPMD)."""
    n, pn, tpc, ppad, nb = cfg["N"], cfg["PN"], cfg["TPC"], cfg["PPAD"], cfg["NB"]
    bs1 = cfg["BS1"]
    in_c, hid, out_c = cfg["IN"], cfg["HID"], cfg["OUT"]
    nsup = (tpc + TSUP - 1) // TSUP
    supers = [list(range(i, min(i + TSUP, tpc))) for i in range(0, tpc, TSUP)]

    x = np.asarray(x, np.float32)
    edge_index = np.asarray(edge_index)
    row = edge_index[0].astype(np.int64)
    col = edge_index[1].astype(np.int64)

    deg = np.bincount(col, minlength=n).astype(np.float32) + 1.0
    dis = 1.0 / np.sqrt(deg)  # [n]

    core = col // pn
    t_of = (col - core * pn) // 128
    si_of = t_of // TSUP
    tl_of = t_of % TSUP
    dls = (col - core * pn) % 128 + tl_of * 128  # dest encoded within super
    b_of = row // bs1
    l1i = (row - b_of * bs1).astype(np.int16)
    shard = row // pn
    l2i = ((shard - 2 * b_of) * ppad + (row - shard * pn)).astype(np.int16)
    norm = (dis[row] * dis[col]).astype(np.float32)

    # counts per (core, super, block, tile-in-super)
    key_t = (((core * nsup) + si_of) * nb + b_of) * TSUP + tl_of
    cnt = np.bincount(key_t, minlength=NCORES * nsup * nb * TSUP).reshape(
        NCORES, nsup, nb, TSUP
    )
    seg = cnt.sum(axis=3).max(axis=0)  # [nsup, nb] max over cores
    seg = ((seg + 127) // 128) * 128

    # global layout: (super, block) contiguous; gather call sizes 256-quantized
    co = np.zeros((nsup, nb), np.int64)
    pos = 0
    for si in range(nsup):
        for b in range(nb):
            co[si][b] = pos
            pos += int(seg[si][b])
    tot = pos
    nidx = seg  # [nsup, nb]
    nidx_call = ((nidx + 255) // 256) * 256
    tot_io = tot + 512  # zero tail so quantized idx reads stay in bounds

    # union per-tile block ranges [jlo, jhi) within each (si, b) segment
    lo = np.cumsum(cnt, axis=3) - cnt  # [c, si, b, tl] start within group
    hi = lo + cnt
    jlo = np.where(cnt > 0, lo // 128, 10 ** 9).min(axis=0)  # [nsup, nb, tl]
    jhi = np.where(cnt > 0, (hi + 127) // 128, -1).max(axis=0)
    have = (cnt.sum(axis=0) > 0)  # [nsup, nb, tl]

    # per-edge positions: sort by (core, si, b, tl) with source row as the
    # secondary key -- slots within a subgroup ascend by table row, so the
    # gather's DMA descriptors access HBM quasi-sequentially (row-buffer
    # locality) instead of randomly.
    order = np.lexsort((row, key_t))
    group_start = np.zeros(NCORES * nsup * nb * TSUP + 1, np.int64)
    np.cumsum(
        np.bincount(key_t, minlength=NCORES * nsup * nb * TSUP),
        out=group_start[1:],
    )
    ranks = np.empty(len(order), np.int64)
    ranks[order] = np.arange(len(order)) - group_start[key_t[order]]
    # start of (c, si, b, tl) slots within the common segment = lo[c,si,b,tl]
    pos_of_edge = (
        co[si_of, b_of]
        + lo[core, si_of, b_of, tl_of]
        + ranks
    )

    in_maps = []
    w1c = np.ascontiguousarray(W1.astype(BF16).reshape(2, 128, hid))
    w2c = np.ascontiguousarray(W2.astype(BF16).reshape(2, 128, out_c))
    b1c = np.ascontiguousarray(
        np.asarray(b1, np.float32).reshape(2, 128).T
    )  # [128, 2]
    b2r = np.ascontiguousarray(np.tile(np.asarray(b2, np.float32)[None, :], (128, 1)))
    iota512 = np.ascontiguousarray(
        np.tile(np.arange(TSUP * 128, dtype=np.float32)[None, :], (128, 1))
    )
    ident = np.eye(128, dtype=np.float32).astype(BF16)

    xt = np.ascontiguousarray(x.astype(BF16))  # plain gather table

    for c in range(NCORES):
        sel = core == c
        p = pos_of_edge[sel]
        idx1 = np.zeros(tot, np.int16)
        idx2 = np.zeros(tot, np.int16)
        dst = np.full(tot, -1.0, np.float32)
        nrm = np.zeros(tot, np.float32)
        idx1[p] = l1i[sel]
        idx2[p] = l2i[sel]
        dst[p] = dls[sel].astype(np.float32)
        nrm[p] = norm[sel]

        idx1 = np.concatenate([idx1, np.zeros(tot_io - tot, np.int16)])
        idx2 = np.concatenate([idx2, np.zeros(tot_io - tot, np.int16)])
        if os.environ.get("GCN_CONST_IDX"):
            idx1[:] = 0
            idx2[:] = 0
        idx1_w = np.tile(np.ascontiguousarray(idx1.reshape(-1, 16).T), (8, 1))
        idx2_w = np.tile(np.ascontiguousarray(idx2.reshape(-1, 16).T), (8, 1))
        dst_w = np.ascontiguousarray(dst.reshape(-1, 128).T)
        nrm_w = np.ascontiguousarray(nrm.reshape(-1, 128).T)

        dis_own = np.ones(ppad, np.float32)
        dis_own[:pn] = dis[c * pn : (c + 1) * pn]
        diso_w = np.ascontiguousarray(dis_own.reshape(tpc, 128).T)

        xself = np.zeros((ppad, in_c), np.float32)
        xself[:pn] = (
            x[c * pn : (c + 1) * pn] * (dis[c * pn : (c + 1) * pn] ** 2)[:, None]
        )
        xself = np.ascontiguousarray(xself).astype(BF16)

        in_maps.append(
            dict(
                xt=xt,
                xself=xself,
                idx1=idx1_w,
                idx2=idx2_w,
                dst=dst_w,
                nrm=nrm_w,
                diso=diso_w,
                w1=w1c,
                w2=w2c,
                b1c=b1c,
                b2=b2r,
                iota=iota512,
                ident=ident,
            )
        )

    meta = dict(
        co=co, nidx=nidx, nidx_call=nidx_call, tot=tot, tot_io=tot_io,
        jlo=jlo, jhi=jhi, have=have, supers=supers, nsup=nsup,
    )
    return in_maps, meta


# ---------------------------------------------------------------- bass build
def build_program(cfg, meta):
    import concourse.mybir as mybir
    import concourse.tile as tile
    from contextlib import ExitStack

    f32 = mybir.dt.float32
    bf16 = mybir.dt.bfloat16
    i16 = mybir.dt.int16
    Alu = mybir.AluOpType
    Act = mybir.ActivationFunctionType

    n, pn, tpc, ppad, nb = cfg["N"], cfg["PN"], cfg["TPC"], cfg["PPAD"], cfg["NB"]
    bs1, bs2 = cfg["BS1"], cfg["BS2"]
    in_c, hid, out_c = cfg["IN"], cfg["HID"], cfg["OUT"]
    co, nidx, nidx_call = meta["co"], meta["nidx"], meta["nidx_call"]
    tot, tot_io = meta["tot"], meta["tot_io"]
    jlo, jhi, have = meta["jlo"], meta["jhi"], meta["have"]
    supers, nsup = meta["supers"], meta["nsup"]

    import concourse.bacc as bacc

    nqueues = int(os.environ.get("GCN_QUEUES", "4"))
    nc = bacc.Bacc(None, num_devices=NCORES, num_swdge_queues=nqueues)

    xt_d = nc.dram_tensor("xt", [n, in_c], bf16, kind="ExternalInput")
    xself_d = nc.dram_tensor("xself", [ppad, in_c], bf16, kind="ExternalInput")
    idx1_d = nc.dram_tensor("idx1", [128, tot_io // 16], i16, kind="ExternalInput")
    idx2_d = nc.dram_tensor("idx2", [128, tot_io // 16], i16, kind="ExternalInput")
    dst_d = nc.dram_tensor("dst", [128, tot // 128], f32, kind="ExternalInput")
    nrm_d = nc.dram_tensor("nrm", [128, tot // 128], f32, kind="ExternalInput")
    diso_d = nc.dram_tensor("diso", [128, tpc], f32, kind="ExternalInput")
    w1_d = nc.dram_tensor("w1", [2, 128, hid], bf16, kind="ExternalInput")
    w2_d = nc.dram_tensor("w2", [2, 128, out_c], bf16, kind="ExternalInput")
    b1c_d = nc.dram_tensor("b1c", [128, 2], f32, kind="ExternalInput")
    b2_d = nc.dram_tensor("b2", [128, out_c], f32, kind="ExternalInput")
    iota_d = nc.dram_tensor("iota", [128, TSUP * 128], f32, kind="ExternalInput")
    ident_d = nc.dram_tensor("ident", [128, 128], bf16, kind="ExternalInput")
    out_d = nc.dram_tensor("out", [ppad, out_c], f32, kind="ExternalOutput")

    tab_d = nc.dram_tensor("tab", [NCORES * ppad, out_c], bf16, addr_space="Shared")

    nreg_cache = {}
    stages = os.environ.get("GCN_STAGES", "all")
    reps = int(os.environ.get("GCN_REPS", "1"))
    maxcols = int(nidx_call.max()) // 128
    max_super_cols = int(nidx.sum(axis=1).max()) // 128
    max_idx_cols = int(
        max(
            (co[si][nb - 1] - co[si][0] + nidx_call[si][nb - 1])
            for si in range(nsup)
        )
    ) // 16

    with tile.TileContext(nc) as tc, ExitStack() as ctx:
        def nreg(v):
            if v not in nreg_cache:
                nreg_cache[v] = nc.gpsimd.to_reg(v)
            return nreg_cache[v]

        cpool = ctx.enter_context(tc.tile_pool(name="const", bufs=1))
        iota_t = cpool.tile([128, TSUP * 128], f32)
        nc.sync.dma_start(iota_t[:], iota_d[:])
        ident_t = cpool.tile([128, 128], bf16)
        nc.sync.dma_start(ident_t[:], ident_d[:])
        w1_t = cpool.tile([128, 2, hid], bf16)
        w2_t = cpool.tile([128, 2, out_c], bf16)
        for k in range(2):
            nc.sync.dma_start(w1_t[:, k, :], w1_d[k])
            nc.sync.dma_start(w2_t[:, k, :], w2_d[k])
        b1c_t = cpool.tile([128, 2], f32)
        nc.sync.dma_start(b1c_t[:], b1c_d[:])
        b2_t = cpool.tile([128, out_c], f32)
        nc.sync.dma_start(b2_t[:], b2_d[:])
        diso_t = cpool.tile([128, tpc], f32)
        nc.sync.dma_start(diso_t[:], diso_d[:])
        diso2_t = cpool.tile([128, tpc], f32)
        nc.vector.tensor_mul(diso2_t[:], diso_t[:], diso_t[:])

        gq = [0]

        def gather_super(si, idx_d, table_d, bs, feat, ipool, mpool):
            """Issue merged idx load + nb gathers for super si. Returns
            (msgs dict, gather insts)."""
            c0 = int(co[si][0])
            ncols = int(co[si][nb - 1] - c0 + nidx_call[si][nb - 1])
            it = ipool.tile([128, max_idx_cols], i16, tag="idx")
            nc.sync.dma_start(
                it[:, : ncols // 16], idx_d[:, c0 // 16 : (c0 + ncols) // 16]
            )
            msgs = {}
            gathers = []
            for b in range(nb):
                ni = int(nidx_call[si][b])
                if ni == 0:
                    continue
                o0 = int(co[si][b]) - c0
                mt = mpool.tile([128, maxcols, feat], bf16, tag="msgs")
                if os.environ.get("GCN_SKIP_GATHER"):
                    msgs[b] = mt
                    continue
                g = nc.gpsimd.dma_gather(
                    mt[:, : ni // 128, :],
                    table_d[b * bs : (b + 1) * bs, :],
                    it[:, o0 // 16 : (o0 + ni) // 16],
                    ni,
                    nreg(ni),
                    feat,
                    elem_step=feat,
                    single_packet=False,
                    queue_num=gq[0] % nqueues,
                )
                gq[0] += 1
                gathers.append(g)
                msgs[b] = mt
            return msgs, gathers

        def load_dstnrm(si, dpool):
            scol0 = int(co[si][0]) // 128
            scols = int(nidx[si].sum()) // 128
            dt_ = dpool.tile([128, max_super_cols], f32, tag="dst")
            nc.sync.dma_start(dt_[:, :scols], dst_d[:, scol0 : scol0 + scols])
            nm_ = dpool.tile([128, max_super_cols], f32, tag="nrm")
            nc.sync.dma_start(nm_[:, :scols], nrm_d[:, scol0 : scol0 + scols])
            return dt_, nm_

        def tile_blocks(si, tl):
            """(b, j, dcol) list for dest tile tl of super si (union range)."""
            out = []
            for b in range(nb):
                if not have[si][b][tl]:
                    continue
                base = (int(co[si][b]) - int(co[si][0])) // 128
                for j in range(int(jlo[si][b][tl]), int(jhi[si][b][tl])):
                    out.append((b, j, base + j))
            return out

        def build_S(spool, iota_ap, dt_, nm_, dcol):
            S = spool.tile([128, 128], bf16, tag="oneh")
            nc.vector.tensor_scalar(
                S[:],
                iota_ap,
                dt_[:, dcol : dcol + 1],
                nm_[:, dcol : dcol + 1],
                Alu.is_equal,
                Alu.mult,
            )
            return S

        def _one_pass():
            # ---------------- layer 1 + W1 + relu + W2, fused per super
            with (
                tc.tile_pool(name="idx1", bufs=3) as ipool,
                tc.tile_pool(name="msgs", bufs=cfg["MSG_BUFS"]) as mpool,
                tc.tile_pool(name="dstp", bufs=3) as dpool,
                tc.tile_pool(name="oneh", bufs=8) as spool,
                tc.tile_pool(name="xsl", bufs=3) as xsp,
                tc.tile_pool(name="mxT", bufs=6) as mxp,
                tc.tile_pool(name="o1T", bufs=8) as o1p,
                tc.tile_pool(name="h2s", bufs=8) as h2p,
                tc.tile_pool(name="psagg", bufs=3, space="PSUM") as pa,
                tc.tile_pool(name="psW", bufs=2, space="PSUM") as pw,
                tc.tile_pool(name="psW2", bufs=1, space="PSUM") as pw2,
                tc.tile_pool(name="psTr", bufs=2, space="PSUM") as ptr,
            ):
                def w_chain(pend):
                    """mx add + PE-transpose + W1 + relu + W2 + h2 transpose
                    for a finished tile (emitted after the NEXT tile's agg
                    matmuls so PE stays fed)."""
                    psf, xl, t = pend
                    mx = mxp.tile([128, in_c], bf16, tag="mxT")
                    nc.vector.tensor_add(mx[:], psf[:], xl)
                    mxT = []
                    for kk in range(2):
                        ptt = ptr.tile([128, 128], bf16, tag="psTr")
                        nc.tensor.transpose(
                            ptt[:], mx[:, kk * 128 : (kk + 1) * 128], ident_t[:]
                        )
                        mt = o1p.tile([128, 128], bf16, tag="mxTs")
                        nc.scalar.copy(mt[:], ptt[:])
                        mxT.append(mt)
                    ps1 = pw.tile([128, 256], f32, tag="psW")
                    o1T = []
                    for m in range(2):
                        for kk in range(2):
                            nc.tensor.matmul(
                                ps1[:, m * 128 : (m + 1) * 128],
                                w1_t[:, kk, m * 128 : (m + 1) * 128],
                                mxT[kk][:],
                                start=(kk == 0),
                                stop=(kk == 1),
                            )
                        o1 = o1p.tile([128, 128], bf16, tag="o1T")
                        nc.scalar.activation(
                            o1[:],
                            ps1[:, m * 128 : (m + 1) * 128],
                            Act.Relu,
                            bias=b1c_t[:, m : m + 1],
                        )
                        o1T.append(o1)
                    ps2 = pw2.tile([128, out_c], f32, tag="psW2")
                    for kk in range(2):
                        nc.tensor.matmul(
                            ps2[:],
                            w2_t[:, kk, :],
                            o1T[kk][:],
                            start=(kk == 0),
                            stop=(kk == 1),
                        )
                    h2T = h2p.tile([128, 128], bf16, tag="h2s")
                    nc.scalar.copy(h2T[:], ps2[:])
                    pst = ptr.tile([128, 128], bf16, tag="psTr")
                    nc.tensor.transpose(pst[:], h2T[:], ident_t[:])
                    h2 = h2p.tile([128, 128], bf16, tag="h2s")
                    nc.vector.tensor_copy(h2[:], pst[:])
                    nc.sync.dma_start(agin_dm[t * 128 : (t + 1) * 128, :], h2[:])

                pend = None
                for si, s in enumerate(supers):
                    msgs, _ = gather_super(si, idx1_d, xt_d, bs1, in_c, ipool, mpool)
                    dt_, nm_ = load_dstnrm(si, dpool)
                    xls = {}
                    for tl, t in enumerate(s):
                        xl = xsp.tile([128, in_c], bf16, tag="xsl")
                        nc.sync.dma_start(
                            xl[:], xself_d[t * 128 : (t + 1) * 128, :]
                        )
                        xls[tl] = xl
                    for tl, t in enumerate(s):
                        blks = tile_blocks(si, tl)
                        assert blks, f"empty tile {si}/{tl}"
                        iota_ap = iota_t[:, tl * 128 : (tl + 1) * 128]
                        if os.environ.get("GCN_SKIP_MM"):
                            psf = None
                        else:
                            psf = pa.tile([128, in_c], f32, tag="psagg")
                        nblk = len(blks)
                        for k, (b, j, dcol) in enumerate(blks):
                            if os.environ.get("GCN_SKIP_S"):
                                continue
                            S = build_S(spool, iota_ap, dt_, nm_, dcol)
                            if os.environ.get("GCN_SKIP_MM"):
                                continue
                            nc.tensor.matmul(
                                psf[:],
                                S[:],
                                msgs[b][:, j, :],
                                start=(k == 0),
                                stop=(k == nblk - 1),
                            )
                        if os.environ.get("GCN_SKIP_MM") or os.environ.get("GCN_SKIP_W"):
                            continue
                        if pend is not None:
                            w_chain(pend)
                        pend = (psf, xls[tl][:], t)
                if pend is not None:
                    w_chain(pend)

            # ---------------- AllGather h2 table
            if stages in ("ag", "all"):
                cc = nc.gpsimd.collective_compute(
                    "AllGather",
                    mybir.AluOpType.bypass,
                    replica_groups=[list(range(NCORES))],
                    ins=[agin_dm[:, :].opt()],
                    outs=[tab_d.ap().opt()],
                )

            # ---------------- layer-2 aggregation + epilogue
            import bass_rust as _br

            if stages == "all":
                with (
                    tc.tile_pool(name="idx2", bufs=3) as ipool2,
                    tc.tile_pool(name="msgs2", bufs=cfg["MSG_BUFS"]) as mpool2,
                    tc.tile_pool(name="dstp2", bufs=3) as dpool2,
                    tc.tile_pool(name="oneh2", bufs=8) as spool2,
                    tc.tile_pool(name="h2own", bufs=10) as hop,
                    tc.tile_pool(name="wtmp", bufs=6) as wtp,
                    tc.tile_pool(name="o2", bufs=6) as o2p,
                    tc.tile_pool(name="psag2", bufs=6, space="PSUM") as pa2,
                ):
                    for si, s in enumerate(supers):
                        msgs, gs = gather_super(
                            si, idx2_d, tab_d, bs2, out_c, ipool2, mpool2
                        )
                        for g in gs:
                            _br.add_dep_helper(
                                g.ins, cc.ins, sync=True, reason="tab after AG"
                            )
                        dt_, nm_ = load_dstnrm(si, dpool2)
                        h2os = {}
                        for tl, t in enumerate(s):
                            h2o = hop.tile([128, out_c], bf16, tag="h2own")
                            nc.sync.dma_start(
                                h2o[:], agin_dm[t * 128 : (t + 1) * 128, :]
                            )
                            h2os[tl] = h2o[:]
                        for tl, t in enumerate(s):
                            blks = tile_blocks(si, tl)
                            iota_ap = iota_t[:, tl * 128 : (tl + 1) * 128]
                            ps = pa2.tile([128, out_c], f32, tag="psag2")
                            nblk = len(blks)
                            for k, (b, j, dcol) in enumerate(blks):
                                S = build_S(spool2, iota_ap, dt_, nm_, dcol)
                                nc.tensor.matmul(
                                    ps[:],
                                    S[:],
                                    msgs[b][:, j, :],
                                    start=(k == 0),
                                    stop=(k == nblk - 1),
                                )
                            wt = wtp.tile([128, out_c], f32, tag="wtmp")
                            nc.vector.scalar_tensor_tensor(
                                wt[:],
                                h2os[tl],
                                diso2_t[:, t : t + 1],
                                b2_t[:],
                                Alu.mult,
                                Alu.add,
                            )
                            o2 = o2p.tile([128, out_c], f32, tag="o2")
                            nc.vector.tensor_add(o2[:], ps[:], wt[:])
                            nc.sync.dma_start(
                                out_d[t * 128 : (t + 1) * 128, :], o2[:]
                            )

        dram = ctx.enter_context(tc.tile_pool(name="dram", bufs=1, space="DRAM"))
        agin_dm = dram.tile([ppad, out_c], bf16)

        for _ in range(reps):
            _one_pass()

        if stages == "dbg":
            nc.gpsimd.dma_start(out_d[:, :], agin_dm[:, :])
        elif stages != "all":
            nc.sync.dma_start(out_d[0:128, :], b2_t[:])

    nc.compile()
    return nc


# ---------------------------------------------------------------- entry point
def kernel(x, edge_index, W1, b1, W2, b2):
    from concourse.bass_utils import run_bass_kernel_spmd

    cfg = CFG
    in_maps, meta = prep_inputs(cfg, x, edge_index, W1, b1, W2, b2)
    nc = build_program(cfg, meta)
    res = run_bass_kernel_spmd(nc, in_maps, core_ids=list(range(NCORES)))
    outs = [r["out"][: cfg["PN"]] for r in res.results]
    return np.concatenate(outs, axis=0).astype(np.float32)

